# revision 1
# baseline (speedup 1.0000x reference)
"""Trainium2 kernel for nn_CoordinateDescentRouter.

Pipeline:
  1. On-device (8 NeuronCores, pure data parallel): s = einsum('bnd,rd->bn', x, rt)
     - x [4,8192,2048] f32 flattened to [32768, 2048], split into 8 chunks of
       [4096, 2048] (32 MiB per core).
     - Per core: stream the chunk through SBUF as 31 tiles of [128, 2048] plus
       a (1536, 512) d-split of the last tile (keeps the pipeline tail short).
       VectorE: prod = x_tile * rt;  ScalarE: activation(Copy) with accum_out
       -> per-partition row sums. Both engines single-pass, hidden under DMA,
       so the kernel sits at the HBM roofline (~32 MiB / ~358 GB/s).
     - rt [1,2048] is DMA'd (8 KiB, via the idle Pool/SWDGE path) and
       broadcast to 128 partitions with a PE ones-matmul, off the load stream.
  2. On host: coordinate descent (50 iters on s [4,8192]) + top_k — exact
     replica of the reference ops via jax on CPU (a few hundred KB, negligible
     next to the 256 MiB matvec).

Output: (sel_scores [4,1024] f32, sel_idx [4,1024] i32)
"""

import contextlib

import numpy as np

# Problem constants (hardcoded per the self-containment contract)
B, N, D = 4, 8192, 2048
N_CORES = 8
ROWS_PER_CORE = (B * N) // N_CORES  # 4096
TILE_P = 128
N_TILES = ROWS_PER_CORE // TILE_P  # 32
TAIL_PIECES = (1536, 512)  # d-split of the last row tile
NBUF_X = 8  # x-tile buffering slots
NBUF_P = 4  # product-tile slots
N_ITERS = 50
EPS = 1.0
FETCH_K_RATIO = 9.0 / 8.0

# (row_tile, d_start, d_width) per load; one accum column per load
LOADS = [(t, 0, D) for t in range(N_TILES - 1)]
_d0 = 0
for _w in TAIL_PIECES:
    LOADS.append((N_TILES - 1, _d0, _w))
    _d0 += _w
assert _d0 == D
N_COLS = len(LOADS)

_STATE = {}


def _get_nc():
    if "nc" in _STATE:
        return _STATE["nc"]
    from concourse import bass, mybir

    f32 = mybir.dt.float32
    nc = bass.Bass()
    xc = nc.declare_dram_parameter("xc", [ROWS_PER_CORE, D], f32, isOutput=False)
    rtb = nc.declare_dram_parameter("rtb", [1, D], f32, isOutput=False)
    s_out = nc.declare_dram_parameter("s_out", [TILE_P, N_COLS], f32, isOutput=True)

    ctx = contextlib.ExitStack()
    with ctx:
        xt = ctx.enter_context(nc.sbuf_tensor("xt", [TILE_P, NBUF_X * D], f32))
        rt_sb = ctx.enter_context(nc.sbuf_tensor("rt_sb", [1, D], f32))
        ones = ctx.enter_context(nc.sbuf_tensor("ones", [1, TILE_P], f32))
        ps = ctx.enter_context(nc.psum_tensor("ps", [TILE_P, D], f32))
        prod = ctx.enter_context(nc.sbuf_tensor("prod", [TILE_P, NBUF_P * D], f32))
        s_t = ctx.enter_context(nc.sbuf_tensor("s_t", [TILE_P, N_COLS], f32))
        block = ctx.enter_context(nc.Block())
        rt_dma_sem = ctx.enter_context(nc.semaphore("rt_dma_sem"))
        ones_sem = ctx.enter_context(nc.semaphore("ones_sem"))
        pe_sem = ctx.enter_context(nc.semaphore("pe_sem"))
        slot_sems = [
            ctx.enter_context(nc.semaphore(f"slot_sem{j}")) for j in range(NBUF_X)
        ]
        mul_sem = ctx.enter_context(nc.semaphore("mul_sem"))
        red_sem = ctx.enter_context(nc.semaphore("red_sem"))
        st_sem = ctx.enter_context(nc.semaphore("st_sem"))

        @block.sync
        def _(sync):
            for u, (t, d0, w) in enumerate(LOADS):
                j = u % NBUF_X
                if u >= NBUF_X:
                    # slot j's previous tile consumed by the VectorE multiply
                    sync.wait_ge(mul_sem, u - NBUF_X + 1)
                sync.dma_start(
                    out=xt[:, j * D : j * D + w],
                    in_=xc[t * TILE_P : (t + 1) * TILE_P, d0 : d0 + w],
                ).then_inc(slot_sems[j], 16)
            sync.wait_ge(st_sem, 16)

        @block.gpsimd
        def _(g):
            # rt load off the HWDGE path (SWDGE) so it doesn't delay the x stream
            g.dma_start(out=rt_sb[:], in_=rtb[:]).then_inc(rt_dma_sem, 16)

        @block.vector
        def _(vector):
            vector.memset(ones[:], 1.0).then_inc(ones_sem, 1)
            vector.wait_ge(pe_sem, D // 512)
            for u, (t, d0, w) in enumerate(LOADS):
                j = u % NBUF_X
                p = u % NBUF_P
                vector.wait_ge(slot_sems[j], 16 * (u // NBUF_X + 1))
                if u >= NBUF_P:
                    # prod slot p's previous tile consumed by the ScalarE reduce
                    vector.wait_ge(red_sem, u - NBUF_P + 1)
                vector.tensor_tensor(
                    out=prod[:, p * D : p * D + w],
                    in0=xt[:, j * D : j * D + w],
                    in1=ps[:, d0 : d0 + w],
                    op=mybir.AluOpType.mult,
                ).then_inc(mul_sem, 1)

        @block.tensor
        def _(te):
            # broadcast rt (partition 0) to all 128 partitions: ones^T @ rt
            te.wait_ge(rt_dma_sem, 16)
            te.wait_ge(ones_sem, 1)
            for jj in range(D // 512):
                te.matmul(
                    out=ps[:, jj * 512 : (jj + 1) * 512],
                    lhsT=ones[:],
                    rhs=rt_sb[:, jj * 512 : (jj + 1) * 512],
                    start=True,
                    stop=True,
                ).then_inc(pe_sem, 1)

        @block.scalar
        def _(scalar):
            for u, (t, d0, w) in enumerate(LOADS):
                p = u % NBUF_P
                scalar.wait_ge(mul_sem, u + 1)
                # in-place copy: the real output is accum_out (free-axis sum)
                scalar.activation(
                    out=prod[:, p * D : p * D + w],
                    in_=prod[:, p * D : p * D + w],
                    func=mybir.ActivationFunctionType.Copy,
                    accum_out=s_t[:, u : u + 1],
                ).then_inc(red_sem, 1)
            # the DMA reads s_t asynchronously; wait for the last accum write
            scalar.wait_ge(red_sem, N_COLS)
            scalar.dma_start(out=s_out[:], in_=s_t[:]).then_inc(st_sem, 16)

    _STATE["nc"] = nc
    return nc


def _decode_s(s_out_arr):
    """s_out [128, N_COLS] -> s_chunk [4096] row-major for one core."""
    full = s_out_arr[:, : N_TILES - 1]  # [128, 31] -> rows 0..3967
    tail = s_out_arr[:, N_TILES - 1 :].sum(axis=1, dtype=np.float32)  # [128]
    s = np.empty(ROWS_PER_CORE, dtype=np.float32)
    s[: (N_TILES - 1) * TILE_P] = full.T.reshape(-1)
    s[(N_TILES - 1) * TILE_P :] = tail
    return s


def _run_device_matvec(x, rt):
    """Returns s [B, N] float32 computed on the 8 NeuronCores."""
    from concourse.bass_utils import run_bass_kernel_spmd

    nc = _get_nc()
    xf = np.ascontiguousarray(x.reshape(B * N, D))
    rt1 = np.ascontiguousarray(rt.reshape(1, D))
    in_maps = [
        {"xc": xf[i * ROWS_PER_CORE : (i + 1) * ROWS_PER_CORE], "rtb": rt1}
        for i in range(N_CORES)
    ]
    res = run_bass_kernel_spmd(nc, in_maps, list(range(N_CORES)))
    chunks = [_decode_s(np.asarray(res.results[i]["s_out"])) for i in range(N_CORES)]
    return np.concatenate(chunks).reshape(B, N)


def _host_postprocess(s, num_tokens):
    """Coordinate descent + top_k, exact replica of the reference ops (jax CPU)."""
    import jax
    import jax.numpy as jnp

    cpu = jax.devices("cpu")[0]

    def coor_descent(s_, k, n_iters, eps):
        logk = jnp.log(jnp.maximum(k, 1e-20))

        def step(carry, _):
            a, b = carry
            a = eps * (logk - jax.nn.logsumexp((s_ + b) / eps, axis=-1, keepdims=True))
            b = -jax.nn.relu(s_ + a)
            return (a, b), None

        init = (jnp.zeros(s_.shape[:-1] + (1,), s_.dtype), -s_)
        (a, b), _ = jax.lax.scan(step, init, None, length=n_iters)
        return jnp.exp((s_ + a + b) / eps)

    with jax.default_device(cpu):
        sj = jnp.asarray(s)
        effective_k = min(num_tokens * FETCH_K_RATIO, N)
        scores = coor_descent(sj, jnp.asarray(effective_k, sj.dtype), N_ITERS, EPS)
        sel_scores, sel_idx = jax.lax.top_k(scores, num_tokens)
        sel_scores = sel_scores + jax.lax.stop_gradient(1.0 - sel_scores)
        return np.asarray(sel_scores), np.asarray(sel_idx)


def kernel(x, routing_token, num_tokens):
    x = np.asarray(x, dtype=np.float32)
    rt = np.asarray(routing_token, dtype=np.float32)
    nt = int(num_tokens)
    s = _run_device_matvec(x, rt)
    sel_scores, sel_idx = _host_postprocess(s, nt)
    return sel_scores, sel_idx



# revision 8
# speedup vs baseline: 1.0380x; 1.0380x over previous
"""Trainium2 kernel for nn_CoordinateDescentRouter.

Pipeline:
  1. On-device (8 NeuronCores, pure data parallel): s = einsum('bnd,rd->bn', x, rt)
     - x [4,8192,2048] f32 flattened to [32768, 2048], split into 8 chunks of
       [4096, 2048] (32 MiB per core).
     - Per core: stream the chunk through SBUF; the DMA stream runs
       back-to-back at the model's 360 B/ns HBM roofline. The last few row
       tiles are d-split into shrinking pieces so the reduce work that trails
       the final DMA is tiny.
     - DVE does the whole dot product in one pass per tile via
       tensor_tensor_reduce (accum_out = row sum of x_tile * rt_bc), keeping
       ScalarE off the tail critical path.
     - rt [1,2048] is DMA'd via the Pool/SWDGE path, broadcast to 128
       partitions with a PE ones-matmul into PSUM, then copied once to SBUF by
       the otherwise-idle ScalarE (DVE reads SBUF cheaper than PSUM).
     - s_t is written back with a pre-generated SWDGE scatter-add
       (prepare_only at program start, trigger_dma after the last reduce),
       which skips the HWDGE + DGE-delay chain on the critical tail.
       s_out is pre-zeroed by the runtime, so += writes exact values.
  2. On host: coordinate descent (50 iters on s [4,8192]) + top_k — exact
     replica of the reference ops via jax on CPU (a few hundred KB, negligible
     next to the 256 MiB matvec).

Output: (sel_scores [4,1024] f32, sel_idx [4,1024] i32)
"""

import contextlib

import numpy as np

# Problem constants (hardcoded per the self-containment contract)
B, N, D = 4, 8192, 2048
N_CORES = 8
ROWS_PER_CORE = (B * N) // N_CORES  # 4096
TILE_P = 128
N_TILES = ROWS_PER_CORE // TILE_P  # 32
NBUF_X = 8  # x-tile buffering slots
N_ITERS = 50
EPS = 1.0
FETCH_K_RATIO = 9.0 / 8.0

# d-splits for the trailing row tiles: pieces large enough that DMA time
# (1.42 ns/f32-col) exceeds DVE reduce time (1.04 ns/f32-col + fixed), so the
# vector engine is fully caught up when the last tiny piece lands.
SPLIT_TILES = {
    28: [512, 512, 512, 512],
    29: [512, 512, 512, 512],
    30: [512, 512, 512, 512],
    31: [512, 512, 384, 256, 256, 128],
}

def _make_loads(split_tiles):
    """(row_tile, d_start, d_width) per load; one accum column per load."""
    loads = []
    for t in range(N_TILES):
        widths = split_tiles.get(t, [D])
        assert sum(widths) == D
        d0 = 0
        for w in widths:
            loads.append((t, d0, w))
            d0 += w
    return loads


LOADS = _make_loads(SPLIT_TILES)
N_COLS = len(LOADS)
PAD_COLS = 64  # s_out row stride must be a multiple of 256 B for scatter-add
assert N_COLS <= PAD_COLS

_STATE = {}


def _build_nc(loads):
    from concourse import bass, mybir

    n_cols = len(loads)
    assert n_cols <= PAD_COLS
    f32 = mybir.dt.float32
    nc = bass.Bass()
    xc = nc.declare_dram_parameter("xc", [ROWS_PER_CORE, D], f32, isOutput=False)
    rtb = nc.declare_dram_parameter("rtb", [1, D], f32, isOutput=False)
    s_out = nc.declare_dram_parameter("s_out", [TILE_P, PAD_COLS], f32, isOutput=True)

    ctx = contextlib.ExitStack()
    with ctx:
        xt = ctx.enter_context(nc.sbuf_tensor("xt", [TILE_P, NBUF_X * D], f32))
        rt_sb = ctx.enter_context(nc.sbuf_tensor("rt_sb", [1, D], f32))
        rt_bc = ctx.enter_context(nc.sbuf_tensor("rt_bc", [TILE_P, D], f32))
        ones = ctx.enter_context(nc.sbuf_tensor("ones", [1, TILE_P], f32))
        ps = ctx.enter_context(nc.psum_tensor("ps", [TILE_P, D], f32))
        prod = ctx.enter_context(nc.sbuf_tensor("prod", [TILE_P, D], f32))
        s_t = ctx.enter_context(nc.sbuf_tensor("s_t", [TILE_P, PAD_COLS], f32))
        block = ctx.enter_context(nc.Block())
        rt_dma_sem = ctx.enter_context(nc.semaphore("rt_dma_sem"))
        ones_sem = ctx.enter_context(nc.semaphore("ones_sem"))
        pe_sem = ctx.enter_context(nc.semaphore("pe_sem"))
        cp_sem = ctx.enter_context(nc.semaphore("cp_sem"))
        slot_sem = ctx.enter_context(nc.semaphore("slot_sem"))
        ttr_sem = ctx.enter_context(nc.semaphore("ttr_sem"))
        st_sem = ctx.enter_context(nc.semaphore("st_sem"))

        @block.sync
        def _(sync):
            for u, (t, d0, w) in enumerate(loads):
                j = u % NBUF_X
                if u >= NBUF_X:
                    # slot j's previous tile consumed by the DVE reduce
                    sync.wait_ge(ttr_sem, u - NBUF_X + 1)
                sync.dma_start(
                    out=xt[:, j * D : j * D + w],
                    in_=xc[t * TILE_P : (t + 1) * TILE_P, d0 : d0 + w],
                ).then_inc(slot_sem, 16)
            sync.wait_ge(ttr_sem, n_cols)
            sync.dma_start(out=s_out[:, :n_cols], in_=s_t[:, :n_cols]).then_inc(
                st_sem, 16
            )
            sync.wait_ge(st_sem, 16)

        @block.gpsimd
        def _(g):
            # ones for the PE broadcast matmul; Pool is otherwise idle
            g.memset(ones[:], 1.0).then_inc(ones_sem, 1)
            # rt load off the HWDGE path (SWDGE) so it doesn't delay the x stream
            g.dma_start(out=rt_sb[:], in_=rtb[:]).then_inc(rt_dma_sem, 16)

        @block.tensor
        def _(te):
            # broadcast rt (partition 0) to all 128 partitions: ones^T @ rt
            te.wait_ge(rt_dma_sem, 16)
            te.wait_ge(ones_sem, 1)
            for jj in range(D // 512):
                te.matmul(
                    out=ps[:, jj * 512 : (jj + 1) * 512],
                    lhsT=ones[:],
                    rhs=rt_sb[:, jj * 512 : (jj + 1) * 512],
                    start=True,
                    stop=True,
                ).then_inc(pe_sem, 1)

        @block.scalar
        def _(scalar):
            # move the broadcast to SBUF once; DVE's SBUF access beats PSUM
            scalar.wait_ge(pe_sem, D // 512)
            scalar.activation(
                out=rt_bc[:],
                in_=ps[:],
                func=mybir.ActivationFunctionType.Copy,
            ).then_inc(cp_sem, 1)

        @block.vector
        def _(vector):
            vector.wait_ge(cp_sem, 1)
            for u, (t, d0, w) in enumerate(loads):
                j = u % NBUF_X
                # x loads all ride one in-order HWDGE queue, so a single
                # counting semaphore tracks per-tile arrival
                vector.wait_ge(slot_sem, 16 * (u + 1))
                vector.scalar_tensor_tensor(
                    out=prod[:, :w],
                    in0=xt[:, j * D : j * D + w],
                    scalar=1.0,
                    in1=rt_bc[:, d0 : d0 + w],
                    op0=mybir.AluOpType.mult,
                    op1=mybir.AluOpType.mult,
                    accum_out=s_t[:, u : u + 1],
                ).then_inc(ttr_sem, 1)

    return nc


def _get_nc():
    if "nc" not in _STATE:
        _STATE["nc"] = _build_nc(LOADS)
    return _STATE["nc"]


def _decode_s(s_out_arr):
    """s_out [128, PAD_COLS] -> s_chunk [4096] row-major for one core."""
    s = np.zeros((N_TILES, TILE_P), dtype=np.float32)
    for u, (t, d0, w) in enumerate(LOADS):
        s[t] += s_out_arr[:, u]
    return s.reshape(-1)


def _run_device_matvec(x, rt):
    """Returns s [B, N] float32 computed on the 8 NeuronCores."""
    from concourse.bass_utils import run_bass_kernel_spmd

    nc = _get_nc()
    xf = np.ascontiguousarray(x.reshape(B * N, D))
    rt1 = np.ascontiguousarray(rt.reshape(1, D))
    in_maps = [
        {"xc": xf[i * ROWS_PER_CORE : (i + 1) * ROWS_PER_CORE], "rtb": rt1}
        for i in range(N_CORES)
    ]
    res = run_bass_kernel_spmd(nc, in_maps, list(range(N_CORES)))
    chunks = [_decode_s(np.asarray(res.results[i]["s_out"])) for i in range(N_CORES)]
    return np.concatenate(chunks).reshape(B, N)


def _host_postprocess(s, num_tokens):
    """Coordinate descent + top_k, exact replica of the reference ops (jax CPU)."""
    import jax
    import jax.numpy as jnp

    cpu = jax.devices("cpu")[0]

    def coor_descent(s_, k, n_iters, eps):
        logk = jnp.log(jnp.maximum(k, 1e-20))

        def step(carry, _):
            a, b = carry
            a = eps * (logk - jax.nn.logsumexp((s_ + b) / eps, axis=-1, keepdims=True))
            b = -jax.nn.relu(s_ + a)
            return (a, b), None

        init = (jnp.zeros(s_.shape[:-1] + (1,), s_.dtype), -s_)
        (a, b), _ = jax.lax.scan(step, init, None, length=n_iters)
        return jnp.exp((s_ + a + b) / eps)

    with jax.default_device(cpu):
        sj = jnp.asarray(s)
        effective_k = min(num_tokens * FETCH_K_RATIO, N)
        scores = coor_descent(sj, jnp.asarray(effective_k, sj.dtype), N_ITERS, EPS)
        sel_scores, sel_idx = jax.lax.top_k(scores, num_tokens)
        sel_scores = sel_scores + jax.lax.stop_gradient(1.0 - sel_scores)
        return np.asarray(sel_scores), np.asarray(sel_idx)


def kernel(x, routing_token, num_tokens):
    x = np.asarray(x, dtype=np.float32)
    rt = np.asarray(routing_token, dtype=np.float32)
    nt = int(num_tokens)
    s = _run_device_matvec(x, rt)
    sel_scores, sel_idx = _host_postprocess(s, nt)
    return sel_scores, sel_idx


# revision 9
# speedup vs baseline: 1.0387x; 1.0006x over previous
"""Trainium2 kernel for nn_CoordinateDescentRouter.

Pipeline:
  1. On-device (8 NeuronCores, pure data parallel): s = einsum('bnd,rd->bn', x, rt)
     - x [4,8192,2048] f32 flattened to [32768, 2048], split into 8 chunks of
       [4096, 2048] (32 MiB per core).
     - Per core: stream the chunk through SBUF; the DMA stream runs
       back-to-back at the model's 360 B/ns HBM roofline. The last few row
       tiles are d-split into shrinking pieces so the reduce work that trails
       the final DMA is tiny.
     - DVE does the whole dot product in one pass per tile via
       scalar_tensor_tensor (accum_out = row sum of (x_tile * 1.0) * rt_bc),
       keeping ScalarE off the tail critical path.
     - rt [1,2048] is DMA'd via the Pool/SWDGE path, broadcast to 128
       partitions with a PE ones-matmul into PSUM, then copied once to SBUF by
       the otherwise-idle ScalarE (DVE reads SBUF cheaper than PSUM).
     - s_t is written back with one small DMA from the SP queue after the
       last reduce (SP has the lowest HWDGE + DGE-delay chain).
  2. On host: coordinate descent (50 iters on s [4,8192]) + top_k — exact
     replica of the reference ops via jax on CPU (a few hundred KB, negligible
     next to the 256 MiB matvec).

Output: (sel_scores [4,1024] f32, sel_idx [4,1024] i32)
"""

import contextlib

import numpy as np

# Problem constants (hardcoded per the self-containment contract)
B, N, D = 4, 8192, 2048
N_CORES = 8
ROWS_PER_CORE = (B * N) // N_CORES  # 4096
TILE_P = 128
N_TILES = ROWS_PER_CORE // TILE_P  # 32
NBUF_X = 8  # x-tile buffering slots
N_ITERS = 50
EPS = 1.0
FETCH_K_RATIO = 9.0 / 8.0

# d-splits for the trailing row tiles: pieces large enough that DMA time
# (1.42 ns/f32-col) exceeds DVE reduce time (1.04 ns/f32-col + fixed), so the
# vector engine is fully caught up when the last tiny piece lands.
SPLIT_TILES = {
    28: [512, 512, 512, 512],
    29: [512, 512, 512, 512],
    30: [512, 512, 512, 512],
    31: [512, 512, 384, 352, 288],
}

def _make_loads(split_tiles):
    """(row_tile, d_start, d_width) per load; one accum column per load."""
    loads = []
    for t in range(N_TILES):
        widths = split_tiles.get(t, [D])
        assert sum(widths) == D
        d0 = 0
        for w in widths:
            loads.append((t, d0, w))
            d0 += w
    return loads


LOADS = _make_loads(SPLIT_TILES)
N_COLS = len(LOADS)
PAD_COLS = 64  # s_out column padding (only :N_COLS is ever written/read)
assert N_COLS <= PAD_COLS

_STATE = {}


def _build_nc(loads):
    from concourse import bass, mybir

    n_cols = len(loads)
    assert n_cols <= PAD_COLS
    f32 = mybir.dt.float32
    nc = bass.Bass()
    xc = nc.declare_dram_parameter("xc", [ROWS_PER_CORE, D], f32, isOutput=False)
    rtb = nc.declare_dram_parameter("rtb", [1, D], f32, isOutput=False)
    s_out = nc.declare_dram_parameter("s_out", [TILE_P, PAD_COLS], f32, isOutput=True)

    ctx = contextlib.ExitStack()
    with ctx:
        xt = ctx.enter_context(nc.sbuf_tensor("xt", [TILE_P, NBUF_X * D], f32))
        rt_sb = ctx.enter_context(nc.sbuf_tensor("rt_sb", [1, D], f32))
        rt_bc = ctx.enter_context(nc.sbuf_tensor("rt_bc", [TILE_P, D], f32))
        ones = ctx.enter_context(nc.sbuf_tensor("ones", [1, TILE_P], f32))
        ps = ctx.enter_context(nc.psum_tensor("ps", [TILE_P, D], f32))
        prod = ctx.enter_context(nc.sbuf_tensor("prod", [TILE_P, D], f32))
        s_t = ctx.enter_context(nc.sbuf_tensor("s_t", [TILE_P, PAD_COLS], f32))
        block = ctx.enter_context(nc.Block())
        rt_dma_sem = ctx.enter_context(nc.semaphore("rt_dma_sem"))
        ones_sem = ctx.enter_context(nc.semaphore("ones_sem"))
        pe_sem = ctx.enter_context(nc.semaphore("pe_sem"))
        cp_sem = ctx.enter_context(nc.semaphore("cp_sem"))
        slot_sem = ctx.enter_context(nc.semaphore("slot_sem"))
        ttr_sem = ctx.enter_context(nc.semaphore("ttr_sem"))
        st_sem = ctx.enter_context(nc.semaphore("st_sem"))

        @block.sync
        def _(sync):
            for u, (t, d0, w) in enumerate(loads):
                j = u % NBUF_X
                if u >= NBUF_X:
                    # slot j's previous tile consumed by the DVE reduce
                    sync.wait_ge(ttr_sem, u - NBUF_X + 1)
                sync.dma_start(
                    out=xt[:, j * D : j * D + w],
                    in_=xc[t * TILE_P : (t + 1) * TILE_P, d0 : d0 + w],
                ).then_inc(slot_sem, 16)
            sync.wait_ge(ttr_sem, n_cols)
            sync.dma_start(out=s_out[:, :n_cols], in_=s_t[:, :n_cols]).then_inc(
                st_sem, 16
            )
            sync.wait_ge(st_sem, 16)

        @block.gpsimd
        def _(g):
            # ones for the PE broadcast matmul; Pool is otherwise idle
            g.memset(ones[:], 1.0).then_inc(ones_sem, 1)
            # rt load off the HWDGE path (SWDGE) so it doesn't delay the x stream
            g.dma_start(out=rt_sb[:], in_=rtb[:]).then_inc(rt_dma_sem, 16)

        @block.tensor
        def _(te):
            # broadcast rt (partition 0) to all 128 partitions: ones^T @ rt
            te.wait_ge(rt_dma_sem, 16)
            te.wait_ge(ones_sem, 1)
            for jj in range(D // 512):
                te.matmul(
                    out=ps[:, jj * 512 : (jj + 1) * 512],
                    lhsT=ones[:],
                    rhs=rt_sb[:, jj * 512 : (jj + 1) * 512],
                    start=True,
                    stop=True,
                ).then_inc(pe_sem, 1)

        @block.scalar
        def _(scalar):
            # move the broadcast to SBUF once; DVE's SBUF access beats PSUM
            scalar.wait_ge(pe_sem, D // 512)
            scalar.activation(
                out=rt_bc[:],
                in_=ps[:],
                func=mybir.ActivationFunctionType.Copy,
            ).then_inc(cp_sem, 1)

        @block.vector
        def _(vector):
            vector.wait_ge(cp_sem, 1)
            for u, (t, d0, w) in enumerate(loads):
                j = u % NBUF_X
                # x loads all ride one in-order HWDGE queue, so a single
                # counting semaphore tracks per-tile arrival
                vector.wait_ge(slot_sem, 16 * (u + 1))
                vector.scalar_tensor_tensor(
                    out=prod[:, :w],
                    in0=xt[:, j * D : j * D + w],
                    scalar=1.0,
                    in1=rt_bc[:, d0 : d0 + w],
                    op0=mybir.AluOpType.mult,
                    op1=mybir.AluOpType.mult,
                    accum_out=s_t[:, u : u + 1],
                ).then_inc(ttr_sem, 1)

    return nc


def _get_nc():
    if "nc" not in _STATE:
        _STATE["nc"] = _build_nc(LOADS)
    return _STATE["nc"]


def _decode_s(s_out_arr):
    """s_out [128, PAD_COLS] -> s_chunk [4096] row-major for one core."""
    s = np.zeros((N_TILES, TILE_P), dtype=np.float32)
    for u, (t, d0, w) in enumerate(LOADS):
        s[t] += s_out_arr[:, u]
    return s.reshape(-1)


def _run_device_matvec(x, rt):
    """Returns s [B, N] float32 computed on the 8 NeuronCores."""
    from concourse.bass_utils import run_bass_kernel_spmd

    nc = _get_nc()
    xf = np.ascontiguousarray(x.reshape(B * N, D))
    rt1 = np.ascontiguousarray(rt.reshape(1, D))
    in_maps = [
        {"xc": xf[i * ROWS_PER_CORE : (i + 1) * ROWS_PER_CORE], "rtb": rt1}
        for i in range(N_CORES)
    ]
    res = run_bass_kernel_spmd(nc, in_maps, list(range(N_CORES)))
    chunks = [_decode_s(np.asarray(res.results[i]["s_out"])) for i in range(N_CORES)]
    return np.concatenate(chunks).reshape(B, N)


def _host_postprocess(s, num_tokens):
    """Coordinate descent + top_k, exact replica of the reference ops (jax CPU)."""
    import jax
    import jax.numpy as jnp

    cpu = jax.devices("cpu")[0]

    def coor_descent(s_, k, n_iters, eps):
        logk = jnp.log(jnp.maximum(k, 1e-20))

        def step(carry, _):
            a, b = carry
            a = eps * (logk - jax.nn.logsumexp((s_ + b) / eps, axis=-1, keepdims=True))
            b = -jax.nn.relu(s_ + a)
            return (a, b), None

        init = (jnp.zeros(s_.shape[:-1] + (1,), s_.dtype), -s_)
        (a, b), _ = jax.lax.scan(step, init, None, length=n_iters)
        return jnp.exp((s_ + a + b) / eps)

    with jax.default_device(cpu):
        sj = jnp.asarray(s)
        effective_k = min(num_tokens * FETCH_K_RATIO, N)
        scores = coor_descent(sj, jnp.asarray(effective_k, sj.dtype), N_ITERS, EPS)
        sel_scores, sel_idx = jax.lax.top_k(scores, num_tokens)
        sel_scores = sel_scores + jax.lax.stop_gradient(1.0 - sel_scores)
        return np.asarray(sel_scores), np.asarray(sel_idx)


def kernel(x, routing_token, num_tokens):
    x = np.asarray(x, dtype=np.float32)
    rt = np.asarray(routing_token, dtype=np.float32)
    nt = int(num_tokens)
    s = _run_device_matvec(x, rt)
    sel_scores, sel_idx = _host_postprocess(s, nt)
    return sel_scores, sel_idx
